# revision 12
# baseline (speedup 1.0000x reference)
"""Distributed Trainium2 kernel for the Attention module (8 NeuronCores).

Sharding:
  - Projections (wq/wk/wv conv1x1): N-sharded — core r computes all 1024 output
    channels for its 256-column slice of the sequence. Weight columns are
    pre-permuted so output-channel block j holds exactly core j's head channels.
  - AllToAll #1..3: redistributes q/k/v from N-sharded to head-sharded
    (core r owns heads {2r, 2r+1}; channels head-major: 64 d-dims per head).
  - Attention: fully local per (batch, head). Softmax without max subtraction
    (scores are bounded: |s| ~ 5), fp32 prob output, bf16 transposed prob for
    the PV matmul (prob itself stays fp32).
  - AllToAll #4: x back to N-sharded; merge conv1x1 computed N-sharded.
"""

import numpy as np

import concourse.bass as bass
import concourse.mybir as mybir
import concourse.tile as tile
from concourse import bacc
from concourse.bass_utils import run_bass_kernel_spmd
from concourse.masks import make_identity

B, F, N, H, D = 2, 1024, 2048, 16, 64
W = 8                 # cores
NS = N // W           # 256   per-core sequence slice
LH = H // W           # 2     local heads
CH = LH * D           # 128   channels per core (head-major)
P = 128               # partitions
KT = F // P           # 8     contraction tiles
MT = F // P           # 8     output-channel tiles
NB = B * NS           # 512   proj/merge free block
f32 = mybir.dt.float32
f32r = mybir.dt.float32r
bf16 = mybir.dt.bfloat16
AF = mybir.ActivationFunctionType

_CACHE = {}


def _build():
    nc = bacc.Bacc("TRN2", target_bir_lowering=False, debug=False, num_devices=W)

    xq = nc.declare_dram_parameter("xq", [B, F, NS], f32, isOutput=False)
    xk = nc.declare_dram_parameter("xk", [B, F, NS], f32, isOutput=False)
    xv = nc.declare_dram_parameter("xv", [B, F, NS], f32, isOutput=False)
    wqT = nc.declare_dram_parameter("wqT", [F, F], f32, isOutput=False)
    wkT = nc.declare_dram_parameter("wkT", [F, F], f32, isOutput=False)
    wvT = nc.declare_dram_parameter("wvT", [F, F], f32, isOutput=False)
    wmT = nc.declare_dram_parameter("wmT", [F, F], f32, isOutput=False)
    bqv = nc.declare_dram_parameter("bqv", [3, F], f32, isOutput=False)
    bmv = nc.declare_dram_parameter("bmv", [F], f32, isOutput=False)
    prob_ext = nc.declare_dram_parameter("prob", [B, LH, N, N], f32, isOutput=True)
    out_ext = nc.declare_dram_parameter("out", [B, F, NS], f32, isOutput=True)

    rg = [list(range(W))]

    with tile.TileContext(nc) as tc:
        with (
            tc.tile_pool(name="consts", bufs=1) as consts,
            tc.tile_pool(name="persist", bufs=1) as persist,
            tc.tile_pool(name="dram", bufs=1, space="DRAM") as dram,
        ):
            ident = consts.tile([P, P], f32)
            make_identity(nc, ident)

            # head-sharded q/k/v (+x), [channel, b, s, ns] with n = s*NS + ns
            q_sb = persist.tile([P, B, W, NS], f32r)
            k_sb = persist.tile([P, B, W, NS], f32r)
            v_sb = persist.tile([P, B, W, NS], f32)
            x_sb = persist.tile([P, B, W, NS], f32)

            b_sb = consts.tile([P, 3, MT], f32)
            nc.sync.dma_start(
                out=b_sb, in_=bqv.ap().rearrange("t (mi p) -> p t mi", p=P)
            )
            bm_sb = consts.tile([P, MT], f32)
            nc.sync.dma_start(out=bm_sb, in_=bmv.ap().rearrange("(mi p) -> p mi", p=P))

            # ---------------- projections + AllToAll ----------------
            with (
                tc.tile_pool(name="wpool", bufs=1) as wpool,
                tc.tile_pool(name="inpool", bufs=2) as inpool,
                tc.tile_pool(name="ppsum", bufs=4, space="PSUM") as ppsum,
                tc.tile_pool(name="pstage", bufs=4) as pstage,
            ):
                gathered = {}
                for t, wT, x_in, dst in (
                    ("q", wqT, xq, q_sb),
                    ("k", wkT, xk, k_sb),
                    ("v", wvT, xv, v_sb),
                ):
                    a2a_in = dram.tile([W, B, CH, NS], f32, name=f"a2a_in_{t}")
                    a2a_out = dram.tile([W, B, CH, NS], f32, name=f"a2a_out_{t}")
                    w_sb = wpool.tile([P, KT, F], f32r, tag="w")
                    nc.gpsimd.dma_start(
                        out=w_sb, in_=wT.ap().rearrange("(ki p) o -> p ki o", p=P)
                    )
                    in_sb = inpool.tile([P, KT, B, NS], f32r, tag="in")
                    for b in range(B):
                        nc.gpsimd.dma_start(
                            out=in_sb[:, :, b],
                            in_=x_in.ap()[b].rearrange("(ki p) ns -> p ki ns", p=P),
                        )
                    bias_col = {"q": 0, "k": 1, "v": 2}[t]
                    for mi in range(MT):
                        ps = ppsum.tile([P, NB], f32, tag="pp")
                        for ki in range(KT):
                            nc.tensor.matmul(
                                ps,
                                w_sb[:, ki, mi * P : (mi + 1) * P],
                                in_sb[:, ki],
                                start=(ki == 0),
                                stop=(ki == KT - 1),
                            )
                        o_sb = pstage.tile([P, B, NS], f32, tag="po")
                        nc.scalar.activation(
                            o_sb,
                            ps,
                            AF.Identity,
                            bias=b_sb[:, bias_col, mi : mi + 1],
                            scale=0.125 if t == "q" else 1.0,
                        )
                        nc.sync.dma_start(
                            out=a2a_in[mi].rearrange("b c ns -> c b ns"),
                            in_=o_sb,
                        )
                    nc.gpsimd.collective_compute(
                        "AllToAll",
                        mybir.AluOpType.bypass,
                        replica_groups=rg,
                        ins=[a2a_in.opt()],
                        outs=[a2a_out.opt()],
                    )
                    gathered[t] = a2a_out
                for t, dst in (("q", q_sb), ("k", k_sb), ("v", v_sb)):
                    eng = nc.sync if t == "v" else nc.gpsimd
                    for b in range(B):
                        eng.dma_start(
                            out=dst[:, b],
                            in_=gathered[t][:, b].rearrange("s c ns -> c s ns"),
                        )

            # ---------------- attention ----------------
            with (
                tc.tile_pool(name="vt", bufs=1) as vtpool,
                tc.tile_pool(name="apool", bufs=3) as apool,
                tc.tile_pool(name="probT", bufs=2) as probTpool,
                tc.tile_pool(name="scps", bufs=4, space="PSUM") as scps,
                tc.tile_pool(name="tpps", bufs=2, space="PSUM") as tpps,
                tc.tile_pool(name="xps", bufs=2, space="PSUM") as xpps,
            ):
                # v^T tiles per (b, head): [m-part, mt, d] bf16
                vts = {}
                for b in range(B):
                    for h in range(LH):
                        vt = vtpool.tile([P, 16, D], bf16, name=f"vt_{b}_{h}")
                        for mt in range(16):
                            tp = tpps.tile([P, D], f32, tag="tp")
                            nc.tensor.transpose(
                                tp,
                                v_sb[
                                    64 * h : 64 * h + 64, b, mt // 2,
                                    (mt % 2) * 128 : (mt % 2) * 128 + 128,
                                ],
                                ident[64 * h : 64 * h + 64, 64 * h : 64 * h + 64],
                            )
                            nc.vector.tensor_copy(vt[:, mt], tp)
                        vts[(b, h)] = vt

                copy_flip = 0
                for b in range(B):
                    for pi in range(W):
                        pTs = {}
                        for h in range(LH):
                            pTs[h] = probTpool.tile(
                                [P, 16, 2 * P], bf16, tag=f"pT{h}", name=f"pT{h}"
                            )
                        for h in range(LH):
                            for half in range(2):
                                nck = 2 * pi + half
                                prob = apool.tile([P, 4, 512], f32, tag="prob")
                                sums = apool.tile([P, 4], f32, tag="sums")
                                qsl = q_sb[
                                    64 * h : 64 * h + 64, b, pi,
                                    half * P : half * P + P,
                                ]
                                for mi4 in range(4):
                                    sc = scps.tile([P, 512], f32, tag="sc")
                                    nc.tensor.matmul(
                                        sc,
                                        qsl,
                                        k_sb[
                                            64 * h : 64 * h + 64, b,
                                            2 * mi4 : 2 * mi4 + 2,
                                        ],
                                        start=True,
                                        stop=True,
                                        tile_position=(64 * h, 0),
                                    )
                                    nc.scalar.activation(
                                        prob[:, mi4],
                                        sc,
                                        AF.Exp,
                                        accum_out=sums[:, mi4 : mi4 + 1],
                                    )
                                ssum = apool.tile([P, 1], f32, tag="ssum")
                                inv = apool.tile([P, 1], f32, tag="inv")
                                nc.vector.reduce_sum(
                                    ssum, sums, axis=mybir.AxisListType.X
                                )
                                nc.vector.reciprocal(inv, ssum)
                                nc.vector.tensor_scalar_mul(prob, prob, inv)
                                nc.sync.dma_start(
                                    out=prob_ext.ap()[
                                        b, h, nck * P : nck * P + P, :
                                    ],
                                    in_=prob,
                                )
                                pv = prob.rearrange("p a b -> p (a b)")
                                for mt in range(16):
                                    tp = tpps.tile([P, P], f32, tag="tp")
                                    nc.tensor.transpose(
                                        tp,
                                        pv[:, mt * P : (mt + 1) * P],
                                        ident,
                                    )
                                    if copy_flip % 2 == 0:
                                        nc.vector.tensor_copy(
                                            pTs[h][:, mt, half * P : half * P + P], tp
                                        )
                                    else:
                                        nc.scalar.copy(
                                            pTs[h][:, mt, half * P : half * P + P], tp
                                        )
                                    copy_flip += 1
                        x_ps = xpps.tile([P, 2 * P], f32, tag="xp")
                        for h in range(LH):
                            for mt in range(16):
                                nc.tensor.matmul(
                                    x_ps[64 * h : 64 * h + 64, :],
                                    vts[(b, h)][:, mt],
                                    pTs[h][:, mt],
                                    start=(mt == 0),
                                    stop=(mt == 15),
                                    tile_position=(0, 64 * h),
                                )
                        nc.scalar.copy(x_sb[:, b, pi], x_ps)

            # ---------------- x AllToAll + merge ----------------
            a2a_in_x = dram.tile([W, B, CH, NS], f32, name="a2a_in_x")
            a2a_out_x = dram.tile([W, B, CH, NS], f32, name="a2a_out_x")
            with (
                tc.tile_pool(name="mpool", bufs=1) as mpool,
                tc.tile_pool(name="mpsum", bufs=4, space="PSUM") as mpsum,
                tc.tile_pool(name="mstage", bufs=4) as mstage,
            ):
                for j in range(W):
                    nc.sync.dma_start(
                        out=a2a_in_x[j].rearrange("b c ns -> c b ns"),
                        in_=x_sb[:, :, j],
                    )
                nc.gpsimd.collective_compute(
                    "AllToAll",
                    mybir.AluOpType.bypass,
                    replica_groups=rg,
                    ins=[a2a_in_x.opt()],
                    outs=[a2a_out_x.opt()],
                )
                xg_sb = mpool.tile([P, KT, B, NS], f32r)
                for b in range(B):
                    nc.gpsimd.dma_start(
                        out=xg_sb[:, :, b],
                        in_=a2a_out_x[:, b].rearrange("ci c ns -> c ci ns"),
                    )
                wm_sb = mpool.tile([P, KT, F], f32r)
                nc.gpsimd.dma_start(
                    out=wm_sb, in_=wmT.ap().rearrange("(ci p) o -> p ci o", p=P)
                )
                out_view = out_ext.ap().rearrange("b (mo p) ns -> mo p b ns", p=P)
                for mo in range(MT):
                    ps = mpsum.tile([P, NB], f32, tag="mp")
                    for ci in range(KT):
                        nc.tensor.matmul(
                            ps,
                            wm_sb[:, ci, mo * P : (mo + 1) * P],
                            xg_sb[:, ci],
                            start=(ci == 0),
                            stop=(ci == KT - 1),
                        )
                    o_sb = mstage.tile([P, B, NS], f32, tag="mo")
                    nc.scalar.activation(
                        o_sb, ps, AF.Identity, bias=bm_sb[:, mo : mo + 1]
                    )
                    nc.sync.dma_start(out=out_view[mo], in_=o_sb)
    nc.compile()
    return nc


def _get_nc():
    if "nc" not in _CACHE:
        _CACHE["nc"] = _build()
    return _CACHE["nc"]


def _perm():
    idx = []
    for r in range(W):
        for j in range(LH):
            idx.extend(i * H + LH * r + j for i in range(D))
    return np.array(idx)


def prep_in_maps(query, key_, value, wq, bq, wk, bk, wv, bv, wm, bm):
    query = np.asarray(query, np.float32)
    key_ = np.asarray(key_, np.float32)
    value = np.asarray(value, np.float32)
    perm = _perm()
    wqT = np.ascontiguousarray(np.asarray(wq, np.float32)[perm].T)
    wkT = np.ascontiguousarray(np.asarray(wk, np.float32)[perm].T)
    wvT = np.ascontiguousarray(np.asarray(wv, np.float32)[perm].T)
    wmT = np.ascontiguousarray(np.asarray(wm, np.float32).T[perm])
    bqv = np.ascontiguousarray(
        np.stack(
            [
                np.asarray(bq, np.float32)[perm] * 0.125,
                np.asarray(bk, np.float32)[perm],
                np.asarray(bv, np.float32)[perm],
            ]
        )
    )
    bmv = np.ascontiguousarray(np.asarray(bm, np.float32))

    in_maps = []
    for r in range(W):
        sl = slice(r * NS, (r + 1) * NS)
        in_maps.append(
            {
                "xq": np.ascontiguousarray(query[:, :, sl]),
                "xk": np.ascontiguousarray(key_[:, :, sl]),
                "xv": np.ascontiguousarray(value[:, :, sl]),
                "wqT": wqT,
                "wkT": wkT,
                "wvT": wvT,
                "wmT": wmT,
                "bqv": bqv,
                "bmv": bmv,
            }
        )
    return in_maps


def kernel(query, key_, value, wq, bq, wk, bk, wv, bv, wm, bm, **kw):
    in_maps = prep_in_maps(query, key_, value, wq, bq, wk, bk, wv, bv, wm, bm)
    nc = _get_nc()
    res = run_bass_kernel_spmd(nc, in_maps, core_ids=list(range(W)), **kw)
    outs = res.results
    prob = np.empty((B, H, N, N), np.float32)
    out = np.empty((B, F, N), np.float32)
    for r in range(W):
        pr = outs[r]["prob"].reshape(B, LH, N, N)
        for j in range(LH):
            prob[:, LH * r + j] = pr[:, j]
        out[:, :, r * NS : (r + 1) * NS] = outs[r]["out"].reshape(B, F, NS)
    return out, prob


# revision 17
# speedup vs baseline: 5.5513x; 5.5513x over previous
"""Distributed Trainium2 kernel for the Attention module (8 NeuronCores). v2

Sharding (same math as v1):
  - N-sharded projections with channel-permuted weights (block j = core j's
    head channels, head-major), fp32r matmuls.
  - Per-batch AllToAlls: qk_b0, qk_b1 (after k proj), v_b0, v_b1 (after v
    proj) so attention on b0 starts after one small collective and the rest
    overlap attention compute.
  - Attention per (b, head): fp32r QK with no-max-sub softmax (scores are
    bounded ~±5), ACT exp (f32r out) + accumulator row-sums, DVE reciprocal +
    in-place normalize, fp32r PE transposes of prob (batched 4/psum-bank),
    DVE psum->sbuf copies casting to bf16, bf16 PV matmul (both heads
    column-packed into one PSUM tile).
  - Per-batch x AllToAll + per-batch merge GEMM overlap the other batch's
    attention; fp32 prob output is exact-normalized f32r values.
"""

import numpy as np

import concourse.bass as bass
import concourse.mybir as mybir
import concourse.tile as tile
from concourse import bacc
from concourse.bass_utils import run_bass_kernel_spmd
from concourse.masks import make_identity

B, F, N, H, D = 2, 1024, 2048, 16, 64
W = 8
NS = N // W           # 256
LH = H // W           # 2
CH = LH * D           # 128
P = 128
KT = F // P           # 8
MT = F // P           # 8
f32 = mybir.dt.float32
f32r = mybir.dt.float32r
bf16 = mybir.dt.bfloat16
AF = mybir.ActivationFunctionType

_CACHE = {}


def _cc(nc, rg, ins_t, outs_t, mock):
    if mock:
        return nc.sync.dma_start(out=outs_t.flatten(), in_=ins_t.flatten())
    return nc.gpsimd.collective_compute(
        "AllToAll",
        mybir.AluOpType.bypass,
        replica_groups=rg,
        ins=[ins_t.opt()],
        outs=[outs_t.opt()],
    )


def _build(repeat=1, trivial=False, mock_cc=False):
    nc = bacc.Bacc("TRN2", target_bir_lowering=False, debug=False, num_devices=W)

    xq = nc.declare_dram_parameter("xq", [B, F, NS], f32, isOutput=False)
    xk = nc.declare_dram_parameter("xk", [B, F, NS], f32, isOutput=False)
    xv = nc.declare_dram_parameter("xv", [B, F, NS], f32, isOutput=False)
    wqT = nc.declare_dram_parameter("wqT", [F, F], f32, isOutput=False)
    wkT = nc.declare_dram_parameter("wkT", [F, F], f32, isOutput=False)
    wvT = nc.declare_dram_parameter("wvT", [F, F], f32, isOutput=False)
    wmT = nc.declare_dram_parameter("wmT", [F, F], f32, isOutput=False)
    bqv = nc.declare_dram_parameter("bqv", [3, F], f32, isOutput=False)
    bmv = nc.declare_dram_parameter("bmv", [F], f32, isOutput=False)
    prob_ext = nc.declare_dram_parameter("prob", [B, LH, N, N], f32, isOutput=True)
    out_ext = nc.declare_dram_parameter("out", [B, F, NS], f32, isOutput=True)

    rg = [list(range(W))]

    if trivial:
        with tile.TileContext(nc) as tc:
            with tc.tile_pool(name="tpool", bufs=1) as tp_:
                t_sb = tp_.tile([P, NS], f32)
                nc.sync.dma_start(out=t_sb, in_=xq.ap()[0, 0:P, :])
                nc.sync.dma_start(out=out_ext.ap()[0, 0:P, :], in_=t_sb)
        nc.compile()
        return nc

    for _rep in range(repeat):
      with tile.TileContext(nc) as tc:
        with (
            tc.tile_pool(name="consts", bufs=1) as consts,
            tc.tile_pool(name="persist", bufs=1) as persist,
            tc.tile_pool(name="dram", bufs=1, space="DRAM") as dram,
        ):
            ident_f = consts.tile([P, P], f32)
            make_identity(nc, ident_f)
            ident = consts.tile([P, P], f32r)
            nc.vector.tensor_copy(ident, ident_f)

            q_sb = persist.tile([P, B, W, NS], f32r)
            k_sb = persist.tile([P, B, W, NS], f32r)
            v_sb = persist.tile([P, B, W, NS], f32)
            x_sb = persist.tile([P, B, W, NS], f32)

            b_sb = consts.tile([P, 3, MT], f32)
            nc.sync.dma_start(
                out=b_sb, in_=bqv.ap().rearrange("t (mi p) -> p t mi", p=P)
            )
            bm_sb = consts.tile([P, MT], f32)
            nc.sync.dma_start(out=bm_sb, in_=bmv.ap().rearrange("(mi p) -> p mi", p=P))

            a2a_qk_in = [
                dram.tile([W, 2, CH, NS], f32, name=f"a2a_qk_in{b}") for b in range(B)
            ]
            a2a_qk_out = [
                dram.tile([W, 2, CH, NS], f32, name=f"a2a_qk_out{b}") for b in range(B)
            ]
            a2a_v_in = [
                dram.tile([W, CH, NS], f32, name=f"a2a_v_in{b}") for b in range(B)
            ]
            a2a_v_out = [
                dram.tile([W, CH, NS], f32, name=f"a2a_v_out{b}") for b in range(B)
            ]
            a2a_x_in = [
                dram.tile([W, CH, NS], f32, name=f"a2a_x_in{b}") for b in range(B)
            ]
            a2a_x_out = [
                dram.tile([W, CH, NS], f32, name=f"a2a_x_out{b}") for b in range(B)
            ]

            # ---------------- projections + per-batch AllToAll ----------------
            with (
                tc.tile_pool(name="wpool", bufs=1) as wpool,
                tc.tile_pool(name="inpool", bufs=2) as inpool,
                tc.tile_pool(name="ppsum", bufs=4, space="PSUM") as ppsum,
                tc.tile_pool(name="pstage", bufs=4) as pstage,
            ):
                for tsel, (t, wT, x_in) in enumerate(
                    (("q", wqT, xq), ("k", wkT, xk), ("v", wvT, xv))
                ):
                    w_f = wpool.tile([P, KT, F], f32, tag="wf")
                    nc.sync.dma_start(
                        out=w_f, in_=wT.ap().rearrange("(ki p) o -> p ki o", p=P)
                    )
                    w_sb = wpool.tile([P, KT, F], f32r, tag="w")
                    nc.vector.tensor_copy(w_sb, w_f)
                    in_f = inpool.tile([P, KT, B, NS], f32, tag="inf", bufs=1)
                    for b in range(B):
                        nc.sync.dma_start(
                            out=in_f[:, :, b],
                            in_=x_in.ap()[b].rearrange("(ki p) ns -> p ki ns", p=P),
                        )
                    in_sb = inpool.tile([P, KT, B, NS], f32r, tag="in", bufs=1)
                    nc.vector.tensor_copy(in_sb, in_f)
                    for mi in range(MT):
                        ps = ppsum.tile([P, B * NS], f32, tag="pp")
                        for ki in range(KT):
                            nc.tensor.matmul(
                                ps,
                                w_sb[:, ki, mi * P : (mi + 1) * P],
                                in_sb[:, ki],
                                start=(ki == 0),
                                stop=(ki == KT - 1),
                            )
                        o_sb = pstage.tile([P, B, NS], f32r, tag="po")
                        nc.scalar.activation(
                            o_sb,
                            ps,
                            AF.Identity,
                            bias=b_sb[:, tsel, mi : mi + 1],
                            scale=0.125 if t == "q" else 1.0,
                        )
                        for b in range(B):
                            if tsel < 2:
                                dst = a2a_qk_in[b][mi, tsel]
                            else:
                                dst = a2a_v_in[b][mi]
                            nc.sync.dma_start(out=dst, in_=o_sb[:, b].bitcast(f32))
                    if tsel == 1:
                        cc_prev = _cc(nc, rg, a2a_qk_in[0], a2a_qk_out[0], mock_cc)
                    if tsel == 2:
                        for ins_t, outs_t in (
                            (a2a_v_in[0], a2a_v_out[0]),
                            (a2a_qk_in[1], a2a_qk_out[1]),
                            (a2a_v_in[1], a2a_v_out[1]),
                        ):
                            cc = _cc(nc, rg, ins_t, outs_t, mock_cc)
                            tile.add_dep_helper(
                                cc.ins, cc_prev.ins, sync=False,
                                reason="keep collective issue order",
                            )
                            cc_prev = cc

            # ---------------- attention + per-batch x A2A + merge ------------
            with (
                tc.tile_pool(name="vt", bufs=1) as vtpool,
                tc.tile_pool(name="apool", bufs=3) as apool,
                tc.tile_pool(name="probT", bufs=2) as probTpool,
                tc.tile_pool(name="mpool", bufs=1) as mpool,
                tc.tile_pool(name="scps", bufs=1, space="PSUM") as scps,
                tc.tile_pool(name="tpps", bufs=2, space="PSUM") as tpps,
                tc.tile_pool(name="xps", bufs=1, space="PSUM") as xpps,
                tc.tile_pool(name="mpsum", bufs=1, space="PSUM") as mpsum,
                tc.tile_pool(name="mstage", bufs=2) as mstage,
            ):
                wm_sb = mpool.tile([P, KT, F], f32r)
                nc.gpsimd.dma_start(
                    out=wm_sb, in_=wmT.ap().rearrange("(ci p) o -> p ci o", p=P)
                )
                prob_dmas = []
                for b in range(B):
                    gathers = [
                        nc.sync.dma_start(
                            out=q_sb[:, b].bitcast(f32),
                            in_=a2a_qk_out[b][:, 0].rearrange("s c ns -> c s ns"),
                        ),
                        nc.sync.dma_start(
                            out=k_sb[:, b].bitcast(f32),
                            in_=a2a_qk_out[b][:, 1].rearrange("s c ns -> c s ns"),
                        ),
                        nc.sync.dma_start(
                            out=v_sb[:, b],
                            in_=a2a_v_out[b].rearrange("s c ns -> c s ns"),
                        ),
                    ]
                    if prob_dmas:
                        anchor = prob_dmas[max(0, len(prob_dmas) - 10)]
                        for g in gathers:
                            tile.add_dep_helper(
                                g.ins, anchor.ins, sync=False,
                                reason="batch-1 gathers behind batch-0 prob writes",
                            )
                    # v^T tiles for this batch
                    vts = []
                    for h in range(LH):
                        vt = vtpool.tile(
                            [P, 16, D], bf16, name=f"vt_{b}_{h}", tag=f"vt{h}", bufs=2
                        )
                        for mt in range(16):
                            tp = tpps.tile([P, D], f32, tag="tp")
                            nc.tensor.transpose(
                                tp,
                                v_sb[
                                    64 * h : 64 * h + 64, b, mt // 2,
                                    (mt % 2) * 128 : (mt % 2) * 128 + 128,
                                ],
                                ident_f[64 * h : 64 * h + 64, 64 * h : 64 * h + 64],
                            )
                            nc.scalar.copy(vt[:, mt], tp)
                        vts.append(vt)

                    for pi in range(W):
                        pTs = []
                        for h in range(LH):
                            pTs.append(
                                probTpool.tile(
                                    [P, 16, 2 * P], bf16, tag=f"pT{h}", name=f"pT{h}"
                                )
                            )
                        for half in range(2):
                            nck = 2 * pi + half
                            probs, sums_t = [], []
                            for h in range(LH):
                                probs.append(
                                    apool.tile(
                                        [P, 4, 512], f32r, tag=f"prob{h}",
                                        name=f"prob{h}", bufs=2,
                                    )
                                )
                                sums_t.append(
                                    apool.tile(
                                        [P, 2], f32, tag=f"sums{h}", name=f"sums{h}"
                                    )
                                )
                            for mi2 in range(2):
                                scs = [
                                    scps.tile(
                                        [P, 2, 512], f32, tag=f"sc{h}", name=f"sc{h}"
                                    )
                                    for h in range(LH)
                                ]
                                for sub in range(2):
                                    mi4 = 2 * mi2 + sub
                                    for h in range(LH):
                                        nc.tensor.matmul(
                                            scs[h][:, sub],
                                            q_sb[
                                                64 * h : 64 * h + 64, b, pi,
                                                half * P : half * P + P,
                                            ],
                                            k_sb[
                                                64 * h : 64 * h + 64, b,
                                                2 * mi4 : 2 * mi4 + 2,
                                            ],
                                            start=True,
                                            stop=True,
                                            tile_position=(64 * h, 0),
                                        )
                                for h in range(LH):
                                    nc.scalar.activation(
                                        probs[h][:, 2 * mi2 : 2 * mi2 + 2],
                                        scs[h],
                                        AF.Exp,
                                        accum_out=sums_t[h][:, mi2 : mi2 + 1],
                                    )
                            for h in range(LH):
                                prob = probs[h]
                                ssum = apool.tile([P, 1], f32, tag="ssum")
                                inv = apool.tile([P, 1], f32, tag="inv")
                                nc.vector.reduce_sum(
                                    ssum, sums_t[h], axis=mybir.AxisListType.X
                                )
                                nc.vector.reciprocal(inv, ssum)
                                nc.vector.tensor_scalar_mul(prob, prob, inv)
                                prob_dmas.append(nc.sync.dma_start(
                                    out=prob_ext.ap()[
                                        b, h, nck * P : nck * P + P, :
                                    ],
                                    in_=prob.bitcast(f32),
                                ))
                                pv = prob.rearrange("p a b -> p (a b)")
                                for mg in range(4):
                                    tp = tpps.tile([P, 4, P], f32r, tag="tp")
                                    for g in range(4):
                                        nc.tensor.transpose(
                                            tp[:, g],
                                            pv[
                                                :,
                                                (4 * mg + g) * P : (4 * mg + g + 1) * P,
                                            ],
                                            ident,
                                        )
                                    dst = pTs[h][
                                        :, 4 * mg : 4 * mg + 4, half * P : half * P + P
                                    ]
                                    nc.vector.tensor_copy(dst, tp)
                        x_ps = xpps.tile([P, 2 * P], f32, tag="xp")
                        for mt in range(16):
                            for h in range(LH):
                                nc.tensor.matmul(
                                    x_ps[64 * h : 64 * h + 64, :],
                                    vts[h][:, mt],
                                    pTs[h][:, mt],
                                    start=(mt == 0),
                                    stop=(mt == 15),
                                    tile_position=(0, 64 * h),
                                )
                        nc.scalar.copy(x_sb[:, b, pi], x_ps)

                    # batch b attention done: A2A x_b + merge_b (overlaps b+1)
                    for j in range(W):
                        nc.sync.dma_start(out=a2a_x_in[b][j], in_=x_sb[:, b, j])
                    _cc(nc, rg, a2a_x_in[b], a2a_x_out[b], mock_cc)
                # merges run after all attention so batch-1 work is not
                # queued behind merge-b0 on the engine streams
                for b in range(B):
                    xg_sb = mpool.tile([P, KT, NS], f32r, name=f"xg{b}", tag="xg", bufs=2)
                    nc.gpsimd.dma_start(
                        out=xg_sb,
                        in_=a2a_x_out[b].rearrange("ci c ns -> c ci ns"),
                    )
                    out_view = out_ext.ap()[b].rearrange("(mo p) ns -> mo p ns", p=P)
                    for mo in range(MT):
                        ps = mpsum.tile([P, NS], f32, tag="mp")
                        for ci in range(KT):
                            nc.tensor.matmul(
                                ps,
                                wm_sb[:, ci, mo * P : (mo + 1) * P],
                                xg_sb[:, ci],
                                start=(ci == 0),
                                stop=(ci == KT - 1),
                            )
                        o_sb = mstage.tile([P, NS], f32, tag="mo")
                        nc.scalar.activation(
                            o_sb, ps, AF.Identity, bias=bm_sb[:, mo : mo + 1]
                        )
                        nc.sync.dma_start(out=out_view[mo], in_=o_sb)
    nc.compile()
    return nc


def _get_nc(repeat=1, trivial=False, mock_cc=False):
    key = ("nc", repeat, trivial, mock_cc)
    if key not in _CACHE:
        _CACHE[key] = _build(repeat, trivial, mock_cc)
    return _CACHE[key]


def _perm():
    idx = []
    for r in range(W):
        for j in range(LH):
            idx.extend(i * H + LH * r + j for i in range(D))
    return np.array(idx)


def prep_in_maps(query, key_, value, wq, bq, wk, bk, wv, bv, wm, bm):
    query = np.asarray(query, np.float32)
    key_ = np.asarray(key_, np.float32)
    value = np.asarray(value, np.float32)
    perm = _perm()
    wqT = np.ascontiguousarray(np.asarray(wq, np.float32)[perm].T)
    wkT = np.ascontiguousarray(np.asarray(wk, np.float32)[perm].T)
    wvT = np.ascontiguousarray(np.asarray(wv, np.float32)[perm].T)
    wmT = np.ascontiguousarray(np.asarray(wm, np.float32).T[perm])
    bqv = np.ascontiguousarray(
        np.stack(
            [
                np.asarray(bq, np.float32)[perm] * 0.125,
                np.asarray(bk, np.float32)[perm],
                np.asarray(bv, np.float32)[perm],
            ]
        )
    )
    bmv = np.ascontiguousarray(np.asarray(bm, np.float32))

    in_maps = []
    for r in range(W):
        sl = slice(r * NS, (r + 1) * NS)
        in_maps.append(
            {
                "xq": np.ascontiguousarray(query[:, :, sl]),
                "xk": np.ascontiguousarray(key_[:, :, sl]),
                "xv": np.ascontiguousarray(value[:, :, sl]),
                "wqT": wqT,
                "wkT": wkT,
                "wvT": wvT,
                "wmT": wmT,
                "bqv": bqv,
                "bmv": bmv,
            }
        )
    return in_maps


def kernel(query, key_, value, wq, bq, wk, bk, wv, bv, wm, bm, **kw):
    in_maps = prep_in_maps(query, key_, value, wq, bq, wk, bk, wv, bv, wm, bm)
    nc = _get_nc()
    res = run_bass_kernel_spmd(nc, in_maps, core_ids=list(range(W)), **kw)
    outs = res.results
    prob = np.empty((B, H, N, N), np.float32)
    out = np.empty((B, F, N), np.float32)
    for r in range(W):
        pr = outs[r]["prob"].reshape(B, LH, N, N)
        for j in range(LH):
            prob[:, LH * r + j] = pr[:, j]
        out[:, :, r * NS : (r + 1) * NS] = outs[r]["out"].reshape(B, F, NS)
    return out, prob
